# revision 22
# baseline (speedup 1.0000x reference)
"""Trainium2 Bass kernel for nn_MultiHeadAttention_47485158424810.

Full-input contract: kernel(**inputs) takes the unsharded numpy inputs and
returns the full [2, 2048, 1024] output.

Sharding (8 cores): core = b*4 + hg
  - data parallel over batch b in {0,1}
  - tensor parallel over 4 head-groups hg (4 heads of 64 dims each -> 256
    output dims per core) by splitting Wq/Wk/Wv rows (column-parallel) and
    Wo columns (row-parallel).  Each core emits a partial [2048, 1024]
    output; the host sums the 4 partials per batch and adds the bias row.

Device-side plan per core (T=2048, K=1024, O=256, 4 heads of s=64), all
matmuls bf16 with fp32 psum accumulation:
  - K bias is dropped entirely (adds a per-row constant to logits ->
    softmax invariant); V bias is folded into a host-side bias row
    (softmax rows sum to 1, so O = P V + bv exactly); only the Q bias is
    applied on device, fused into the Q psum eviction.
  - per 512-token chunk: project Q^T,K^T ([o, t] layout) and V (natural
    [t, o] with a ones column per head that makes the AV matmul also
    accumulate softmax denominators for free).
  - attention per (head pair, chunk): S^T tiles for both heads in one
    [128, 1024] psum via row-group-packed matmuls; exp on ACT (1/sqrt(K)
    folded into the activation scale; ACT's table holds exp+copy so it
    never swaps); causal masking via a bf16 0/1 mask with fully-masked
    columns skipped; AV on PE with V stationary.
  - normalization: denominator row -> fast DVE reciprocal -> gpsimd
    partition_broadcast (no psum, no PE) -> DVE multiply evicts the psum
    O rows straight into the normalized bf16 oT tile.
  - output projection per 128-token tile, evicted via DVE/ACT into an
    SBUF stage and DMA'd to DRAM.
  - emission weaves proj(c+1) and outproj(c-1) units into the attention
    r-loop so the PE has fillers while ACT (the attention-phase
    bottleneck) grinds through exp.
"""

import os
import sys

import numpy as np

for _p in ("/root/.axon_site/_ro/trn_rl_repo", "/opt/trn_rl_repo"):
    if os.path.isdir(_p) and _p not in sys.path:
        sys.path.append(_p)

import ml_dtypes

import concourse.bass as bass
import concourse.tile as tile
from concourse import bacc, mybir
from concourse.bass_utils import run_bass_kernel_spmd

B, T, K, H = 2, 2048, 1024, 16
NCORES = 8
O = 256  # head-group width per core (4 heads x 64)
S = 64  # head dim
HPC = 4  # heads per core
F32 = mybir.dt.float32
BF16 = mybir.dt.bfloat16
AF = mybir.ActivationFunctionType
ALU = mybir.AluOpType
NPBF16 = ml_dtypes.bfloat16

_CACHE = {}


def _build_body(nc, tc, d, loop_n=0):
    if loop_n:
        with tc.For_i(0, loop_n, 1):
            with tc.tile_pool(name="consts", bufs=1) as consts, \
                 tc.tile_pool(name="persist", bufs=1) as persist:
                _build_inner(nc, tc, d, consts, persist)
        return
    with tc.tile_pool(name="consts", bufs=1) as consts, \
         tc.tile_pool(name="persist", bufs=1) as persist:
        _build_inner(nc, tc, d, consts, persist)


def _build_inner(nc, tc, d, consts, persist):
    f32 = F32
    x_d, wq_d, wk_d, wv_d, wo_d, bq_d, y_d = (
        d["x"], d["wqT"], d["wkT"], d["wvT"], d["woT"], d["bq"], d["y"],
    )
    x3 = x_d.rearrange("(kk p) t -> p kk t", p=128)

    def load_wT(ap_d, prefix, eng):
        """One DMA for all 8 contraction slices: [128, 8 x O] with slice kk
        at columns kk*O..(kk+1)*O.  Returns per-slice column views."""
        big = consts.tile([128, 8 * O], BF16, name=f"{prefix}_all")
        eng.dma_start(big.rearrange("p (kk o) -> p kk o", o=O),
                      ap_d.rearrange("(kk p) o -> p kk o", p=128))
        return [big[:, kk * O:(kk + 1) * O] for kk in range(8)]

    # Causal mask first on gpsimd (needed by the first diagonal exp).
    # After column trimming, the partially-masked region of a diagonal
    # tile is always the first 128 columns of its trimmed view with the
    # same triangular predicate for every m.  [128, 2x128] bf16, twin
    # halves for the head pair.
    trimask = consts.tile([128, 256], BF16, name="trimask")
    nc.gpsimd.memset(trimask, 1.0)
    tm3 = trimask.rearrange("p (e j) -> p e j", e=2)
    nc.gpsimd.affine_select(
        out=tm3, in_=tm3, pattern=[[0, 2], [1, 128]],
        compare_op=ALU.is_ge, fill=0.0, base=0, channel_multiplier=-1)

    # Weights on HWDGE queues (sync/scalar/vector) — gpsimd SWDGE dispatch
    # is ~1.7us per DMA and would serialize the startup.
    wq_sb = load_wT(wq_d, "wq", nc.sync)
    wk_sb = load_wT(wk_d, "wk", nc.scalar)
    wv_sb = load_wT(wv_d, "wv", nc.scalar)
    wo_sb = []
    for oc in range(2):
        t_ = consts.tile([128, K], BF16, name=f"wo{oc}")
        nc.gpsimd.dma_start(t_, wo_d[oc * 128:(oc + 1) * 128, :])
        wo_sb.append(t_)

    bq_sb = consts.tile([128, 2], f32, name="bq_sb")
    nc.gpsimd.dma_start(bq_sb, bq_d.rearrange("(c p) -> p c", p=128))

    # persistent activations
    qT = [persist.tile([128, T], BF16, name=f"qT{oc}") for oc in range(2)]
    kT = [persist.tile([128, T], BF16, name=f"kT{oc}") for oc in range(2)]
    oT = [persist.tile([128, T], BF16, name=f"oT{oc}") for oc in range(2)]
    # V natural layout, per t_k tile: 4 heads x (64 dims + ones col)
    vv = [persist.tile([128, HPC * (S + 1)], BF16, name=f"v{i}")
          for i in range(T // 128)]
    ones4 = consts.tile([128, HPC], BF16, name="ones4")
    nc.vector.memset(ones4, 1.0)
    for i in range(T // 128):
        nc.vector.tensor_copy(vv[i][:, S::S + 1], ones4)

    # ACT exp-table preload off the critical path: tiny dummy exp early.
    scratch = consts.tile([1, 8], f32, name="scratch")
    nc.vector.memset(scratch, 0.0)

    inv_scale = 1.0 / float(np.sqrt(K))

    def gen_outp(c, ps_pool, ys_pool, ps_tag="ps", tail=False):
        """4 units: one 128-token output-projection tile each."""
        for i in range(4 * c, 4 * c + 4):
            ys = ys_pool.tile([128, K], f32, name="ystg", tag="ystg")
            for jc in range(2):
                py = ps_pool.tile([128, 512], f32, name="py", tag=ps_tag)
                for occ in range(2):
                    nc.tensor.matmul(
                        py,
                        oT[occ][:, i * 128:(i + 1) * 128],
                        wo_sb[occ][:, jc * 512:(jc + 1) * 512],
                        start=(occ == 0), stop=(occ == 1))
                if jc == 0:
                    nc.vector.tensor_copy(ys[:, 0:512], py)
                else:
                    nc.scalar.copy(ys[:, 512:1024], py)
            # mid-kernel: keep DMA dispatch off the ACT queue (exp stream);
            # tail: use all three queues to drain fast.
            engs = ((nc.sync, nc.gpsimd, nc.scalar) if tail
                    else (nc.sync, nc.gpsimd))
            engs[i % len(engs)].dma_start(y_d[i * 128:(i + 1) * 128, :], ys)
            yield

    with tc.tile_pool(name="xTp", bufs=2) as xT_p, \
         tc.tile_pool(name="ppr", bufs=2, space="PSUM") as ppr_p, \
         tc.tile_pool(name="pss", bufs=2, space="PSUM") as pss_p, \
         tc.tile_pool(name="pso", bufs=1, space="PSUM") as pso_p, \
         tc.tile_pool(name="ptile", bufs=4) as pt_p, \
         tc.tile_pool(name="rrp", bufs=2) as rr_p, \
         tc.tile_pool(name="rbp", bufs=2) as rb_p, \
         tc.tile_pool(name="ystg", bufs=2) as ystg_p:

        xT_tiles = {}

        def load_x(c):
            xt = xT_p.tile([128, 8 * 512], BF16, name=f"xT{c}", tag="xT")
            x4 = xt.rearrange("p (kk t) -> p kk t", t=512)
            src = x3[:, :, c * 512:(c + 1) * 512]
            nc.sync.dma_start(x4[:, 0:4, :], src[:, 0:4, :])
            nc.gpsimd.dma_start(x4[:, 4:8, :], src[:, 4:8, :])
            xT_tiles[c] = xt

        def qk_unit(c, oc, w_sb, dest, bias):
            xt = xT_tiles[c]
            ps = ppr_p.tile([128, 512], f32, name="ps_qk", tag="ps")
            for kk in range(8):
                nc.tensor.matmul(
                    ps,
                    w_sb[kk][:, oc * 128:(oc + 1) * 128],
                    xt[:, kk * 512:(kk + 1) * 512],
                    start=(kk == 0), stop=(kk == 7))
            dst = dest[oc][:, c * 512:(c + 1) * 512]
            if bias is not None:
                nc.vector.tensor_scalar_add(dst, ps, bias[:, oc:oc + 1])
            else:
                nc.vector.tensor_copy(dst, ps)

        def v_unit(c, a):
            xt = xT_tiles[c]
            ps = ppr_p.tile([128, O], f32, name="ps_v", tag="ps")
            for kk in range(8):
                nc.tensor.matmul(
                    ps,
                    xt[:, kk * 512 + a * 128:kk * 512 + (a + 1) * 128],
                    wv_sb[kk],
                    start=(kk == 0), stop=(kk == 7))
            for h in range(HPC):
                nc.vector.tensor_copy(
                    vv[c * 4 + a][:, h * (S + 1):h * (S + 1) + S],
                    ps[:, h * S:(h + 1) * S])

        def gen_proj(c):
            """8 units in dependency-useful order: Q oc0, K oc0, V a0..a3,
            Q oc1, K oc1 — everything attn(c, 0) reads comes first, the oc1
            projections (needed only mid-chunk by attn(c, 1)) last.
            Emission order IS dependency order in Tile: a unit must be
            emitted before any unit that reads its output."""
            qk_unit(c, 0, wq_sb, qT, bq_sb)
            yield
            qk_unit(c, 0, wk_sb, kT, None)
            yield
            for a in range(4):
                v_unit(c, a)
                yield
            qk_unit(c, 1, wq_sb, qT, bq_sb)
            yield
            qk_unit(c, 1, wk_sb, kT, None)
            yield

        tm3v = trimask.rearrange("p (e j) -> p e j", e=2)

        def gen_attn(c, oc):
            """Software-pipelined r-loop: S(r)+exp(r) then AV(r-1)."""
            nr = 4 * (c + 1)
            po = [pso_p.tile([S + 1, 512], f32, name=f"po{e}",
                             tag=f"po{e}") for e in range(2)]
            pts = {}

            def s_exp(r):
                m = r - 4 * c
                j0 = 128 * m if m > 0 else 0
                ps = pss_p.tile([128, 1024], f32, name="ps_s", tag="pss")
                for e in range(2):
                    hb = e * 64
                    nc.tensor.matmul(
                        ps[:, e * 512 + j0:(e + 1) * 512],
                        kT[oc][hb:hb + 64, r * 128:(r + 1) * 128],
                        qT[oc][hb:hb + 64, c * 512 + j0:(c + 1) * 512],
                        start=True, stop=True)
                pt = pt_p.tile([128, 1024], BF16, name="pt_exp", tag="pt")
                ps3 = ps.rearrange("p (e j) -> p e j", e=2)[:, :, j0:]
                pt3 = pt.rearrange("p (e j) -> p e j", e=2)[:, :, j0:]
                nc.scalar.activation(pt3, ps3, AF.Exp, scale=inv_scale)
                if m >= 0:
                    nc.vector.tensor_mul(
                        pt3[:, :, 0:128], pt3[:, :, 0:128], tm3v)
                pts[r] = (pt, j0)

            def av(r):
                pt, j0 = pts.pop(r)
                for e in range(2):
                    h = 2 * oc + e
                    nc.tensor.matmul(
                        po[e][:, j0:],
                        vv[r][:, h * (S + 1):(h + 1) * (S + 1)],
                        pt[:, e * 512 + j0:(e + 1) * 512],
                        start=(r == 0), stop=(r == nr - 1))

            s_exp(0)
            yield
            for r in range(1, nr):
                s_exp(r)
                av(r - 1)
                yield
            av(nr - 1)
            # normalize + evict straight into oT (bf16)
            for e in range(2):
                rr = rr_p.tile([1, 512], f32, name="rr", tag=f"rr{e}")
                # NOT reciprocal_approx_fast: the custom-DVE op reads the
                # wrong partition for a PSUM src with nonzero base (HW-
                # verified); plain InstReciprocal handles it correctly.
                nc.vector.reciprocal(rr, po[e][S:S + 1, :])
                rb = rb_p.tile([64, 512], f32, name="rb", tag=f"rb{e}")
                nc.gpsimd.partition_broadcast(rb, rr)
                hb = e * 64
                nc.vector.tensor_mul(
                    oT[oc][hb:hb + 64, c * 512:(c + 1) * 512],
                    po[e][0:S, :], rb)
                yield

        _SENT = object()

        def weave(backbone, fillers, every=3):
            """Emit backbone units, inserting one filler unit (round-robin)
            every `every` backbone units, then drain filler leftovers."""
            n = 0
            fi = 0
            for _ in backbone:
                n += 1
                if n % every == 0 and fillers:
                    fi %= len(fillers)
                    if next(fillers[fi], _SENT) is _SENT:
                        fillers.pop(fi)
                    else:
                        fi += 1
            for f in fillers:
                for _ in f:
                    pass

        # table preload: dummy exp while weights/x load
        nc.scalar.activation(scratch, scratch, AF.Exp, scale=1.0)

        load_x(0)
        g0 = gen_proj(0)
        for _ in range(6):
            next(g0)  # Q oc0, K oc0, V a0..a3 — all attn(0, 0) reads
        load_x(1)
        for c in range(4):
            if 1 <= c <= 2:
                load_x(c + 1)
            fillers = []
            n_fill = 0
            if c == 0:
                fillers.append(g0)  # Q oc1, K oc1 for attn(0, 1)
                n_fill += 2
            if c + 1 < 4:
                fillers.append(gen_proj(c + 1))
                n_fill += 8
            if c > 0:
                fillers.append(gen_outp(c - 1, ppr_p, ystg_p))
                n_fill += 4
            n_back = 2 * (4 * (c + 1) + 2)

            def backbone(c=c):
                yield from gen_attn(c, 0)
                yield from gen_attn(c, 1)

            weave(backbone(), fillers, every=max(1, n_back // (n_fill + 1)))

    # Last chunk's output projection runs after the attention pools close:
    # 4 psum banks + a deeper stage ring drain the last 4 tiles with
    # maximum overlap.
    with tc.tile_pool(name="tailps", bufs=4, space="PSUM") as tail_ps, \
         tc.tile_pool(name="tailys", bufs=3) as tail_ys:
        for _ in gen_outp(3, tail_ps, tail_ys, ps_tag="tps", tail=True):
            pass


def build_program(loop_n=0):
    nc = bacc.Bacc("TRN2", target_bir_lowering=False, debug=False,
                   num_devices=NCORES)
    d = {
        "x": nc.dram_tensor("xT", [K, T], BF16, kind="ExternalInput").ap(),
        "wqT": nc.dram_tensor("wqT", [K, O], BF16, kind="ExternalInput").ap(),
        "wkT": nc.dram_tensor("wkT", [K, O], BF16, kind="ExternalInput").ap(),
        "wvT": nc.dram_tensor("wvT", [K, O], BF16, kind="ExternalInput").ap(),
        "woT": nc.dram_tensor("woT", [O, K], BF16, kind="ExternalInput").ap(),
        "bq": nc.dram_tensor("bq", [O], F32, kind="ExternalInput").ap(),
        "y": nc.dram_tensor("y", [T, K], F32, kind="ExternalOutput").ap(),
    }
    with tile.TileContext(nc) as tc:
        _build_body(nc, tc, d, loop_n=loop_n)
    nc.compile()
    return nc


def _get_program():
    if "nc" not in _CACHE:
        _CACHE["nc"] = build_program()
    return _CACHE["nc"]


def make_in_maps(x, Wq_w, Wk_w, Wv_w, Wo_w, Wq_b, Wk_b, Wv_b):
    in_maps = []
    for core in range(NCORES):
        b, hg = divmod(core, 4)
        sl = slice(hg * O, (hg + 1) * O)
        in_maps.append({
            "xT": np.ascontiguousarray(x[b].T).astype(NPBF16),
            "wqT": np.ascontiguousarray(Wq_w[sl, :].T).astype(NPBF16),
            "wkT": np.ascontiguousarray(Wk_w[sl, :].T).astype(NPBF16),
            "wvT": np.ascontiguousarray(Wv_w[sl, :].T).astype(NPBF16),
            "woT": np.ascontiguousarray(Wo_w[:, sl].T).astype(NPBF16),
            "bq": np.ascontiguousarray(Wq_b[sl], np.float32),
        })
    return in_maps


def _combine(results, Wv_b, Wo_w, Wo_b):
    # V bias passes through softmax exactly (rows sum to 1): add its
    # projected row on the host together with the output bias.
    bias_row = (np.asarray(Wo_b, np.float32)
                + np.asarray(Wv_b, np.float32) @ np.asarray(Wo_w, np.float32).T)
    y = np.empty((B, T, K), np.float32)
    for b in range(B):
        acc = results[b * 4]["y"].copy()
        for hg in range(1, 4):
            acc += results[b * 4 + hg]["y"]
        y[b] = acc + bias_row
    return y


def kernel(x, Wq_w, Wq_b, Wk_w, Wk_b, Wv_w, Wv_b, Wo_w, Wo_b):
    x = np.asarray(x, np.float32)
    nc = _get_program()
    in_maps = make_in_maps(x, np.asarray(Wq_w), np.asarray(Wk_w),
                           np.asarray(Wv_w), np.asarray(Wo_w),
                           np.asarray(Wq_b), np.asarray(Wk_b),
                           np.asarray(Wv_b))
    out = run_bass_kernel_spmd(nc, in_maps, list(range(NCORES)))
    return _combine(out.results, Wv_b, Wo_w, Wo_b)


# revision 24
# speedup vs baseline: 1.1807x; 1.1807x over previous
"""Trainium2 Bass kernel for nn_MultiHeadAttention_47485158424810.

Full-input contract: kernel(**inputs) takes the unsharded numpy inputs and
returns the full [2, 2048, 1024] output.

Sharding (8 cores): core = b*4 + hg
  - data parallel over batch b in {0,1}
  - tensor parallel over 4 head-groups hg (4 heads of 64 dims each -> 256
    output dims per core) by splitting Wq/Wk/Wv rows (column-parallel) and
    Wo columns (row-parallel).  Each core emits a partial [2048, 1024]
    output; the host sums the 4 partials per batch and adds the bias row.

Device-side plan per core (T=2048, K=1024, O=256, 4 heads of s=64), all
matmuls bf16 with fp32 psum accumulation:
  - K bias is dropped entirely (adds a per-row constant to logits ->
    softmax invariant); V bias is folded into a host-side bias row
    (softmax rows sum to 1, so O = P V + bv exactly); only the Q bias is
    applied on device, fused into the Q psum eviction.
  - per 512-token chunk: project Q^T,K^T ([o, t] layout) and V (natural
    [t, o] with a ones column per head that makes the AV matmul also
    accumulate softmax denominators for free).
  - attention per (head pair, chunk): S^T tiles for both heads in one
    [128, 1024] psum via row-group-packed matmuls; exp on ACT (1/sqrt(K)
    folded into the activation scale; ACT's table holds exp+copy so it
    never swaps); causal masking via a bf16 0/1 mask with fully-masked
    columns skipped; AV on PE with V stationary.
  - normalization: denominator row -> fast DVE reciprocal -> gpsimd
    partition_broadcast (no psum, no PE) -> DVE multiply evicts the psum
    O rows straight into the normalized bf16 oT tile.
  - output projection per 128-token tile, evicted via DVE/ACT into an
    SBUF stage and DMA'd to DRAM.
  - emission weaves proj(c+1) and outproj(c-1) units into the attention
    r-loop so the PE has fillers while ACT (the attention-phase
    bottleneck) grinds through exp.
"""

import os
import sys

import numpy as np

for _p in ("/root/.axon_site/_ro/trn_rl_repo", "/opt/trn_rl_repo"):
    if os.path.isdir(_p) and _p not in sys.path:
        sys.path.append(_p)

import ml_dtypes

import concourse.bass as bass
import concourse.tile as tile
from concourse import bacc, mybir
from concourse.bass_utils import run_bass_kernel_spmd

B, T, K, H = 2, 2048, 1024, 16
NCORES = 8
O = 256  # head-group width per core (4 heads x 64)
S = 64  # head dim
HPC = 4  # heads per core
F32 = mybir.dt.float32
BF16 = mybir.dt.bfloat16
AF = mybir.ActivationFunctionType
ALU = mybir.AluOpType
NPBF16 = ml_dtypes.bfloat16

_CACHE = {}


def _build_body(nc, tc, d, loop_n=0):
    if loop_n:
        with tc.For_i(0, loop_n, 1):
            with tc.tile_pool(name="consts", bufs=1) as consts, \
                 tc.tile_pool(name="persist", bufs=1) as persist:
                _build_inner(nc, tc, d, consts, persist)
        return
    with tc.tile_pool(name="consts", bufs=1) as consts, \
         tc.tile_pool(name="persist", bufs=1) as persist:
        _build_inner(nc, tc, d, consts, persist)


def _build_inner(nc, tc, d, consts, persist):
    f32 = F32
    x_d, wq_d, wk_d, wv_d, wo_d, bq_d, y_d = (
        d["x"], d["wqT"], d["wkT"], d["wvT"], d["woT"], d["bq"], d["y"],
    )
    x3 = x_d.rearrange("(kk p) t -> p kk t", p=128)

    def load_wT(ap_d, prefix, eng):
        """One DMA for all 8 contraction slices: [128, 8 x O] with slice kk
        at columns kk*O..(kk+1)*O.  Returns per-slice column views."""
        big = consts.tile([128, 8 * O], BF16, name=f"{prefix}_all")
        eng.dma_start(big.rearrange("p (kk o) -> p kk o", o=O),
                      ap_d.rearrange("(kk p) o -> p kk o", p=128))
        return [big[:, kk * O:(kk + 1) * O] for kk in range(8)]

    # Causal mask first on gpsimd (needed by the first diagonal exp).
    # After column trimming, the partially-masked region of a diagonal
    # tile is always the first 128 columns of its trimmed view with the
    # same triangular predicate for every m.  [128, 2x128] bf16, twin
    # halves for the head pair.
    trimask = consts.tile([128, 256], BF16, name="trimask")
    nc.gpsimd.memset(trimask, 1.0)
    tm3 = trimask.rearrange("p (e j) -> p e j", e=2)
    nc.gpsimd.affine_select(
        out=tm3, in_=tm3, pattern=[[0, 2], [1, 128]],
        compare_op=ALU.is_ge, fill=0.0, base=0, channel_multiplier=-1)

    # Weights on HWDGE queues (sync/scalar/vector) — gpsimd SWDGE dispatch
    # is ~1.7us per DMA and would serialize the startup.
    wq_sb = load_wT(wq_d, "wq", nc.sync)
    wk_sb = load_wT(wk_d, "wk", nc.scalar)
    wv_sb = load_wT(wv_d, "wv", nc.scalar)
    wo_sb = []
    for oc in range(2):
        t_ = consts.tile([128, K], BF16, name=f"wo{oc}")
        nc.gpsimd.dma_start(t_, wo_d[oc * 128:(oc + 1) * 128, :])
        wo_sb.append(t_)

    bq_sb = consts.tile([128, 2], f32, name="bq_sb")
    nc.gpsimd.dma_start(bq_sb, bq_d.rearrange("(c p) -> p c", p=128))

    # persistent activations
    qT = [persist.tile([128, T], BF16, name=f"qT{oc}") for oc in range(2)]
    kT = [persist.tile([128, T], BF16, name=f"kT{oc}") for oc in range(2)]
    oT = [persist.tile([128, T], BF16, name=f"oT{oc}") for oc in range(2)]
    # V natural layout, per t_k tile: 4 heads x (64 dims + ones col)
    vv = [persist.tile([128, HPC * (S + 1)], BF16, name=f"v{i}")
          for i in range(T // 128)]
    ones4 = consts.tile([128, HPC], BF16, name="ones4")
    nc.vector.memset(ones4, 1.0)
    for i in range(T // 128):
        nc.vector.tensor_copy(vv[i][:, S::S + 1], ones4)

    # ACT exp-table preload off the critical path: tiny dummy exp early.
    scratch = consts.tile([1, 8], f32, name="scratch")
    nc.vector.memset(scratch, 0.0)

    inv_scale = 1.0 / float(np.sqrt(K))

    def gen_outp(c, ps_pool, ys_pool, ps_tag="ps", tail=False):
        """4 units: one 128-token output-projection tile each."""
        for i in range(4 * c, 4 * c + 4):
            ys = ys_pool.tile([128, K], f32, name="ystg", tag="ystg")
            for jc in range(2):
                py = ps_pool.tile([128, 512], f32, name="py", tag=ps_tag)
                for occ in range(2):
                    nc.tensor.matmul(
                        py,
                        oT[occ][:, i * 128:(i + 1) * 128],
                        wo_sb[occ][:, jc * 512:(jc + 1) * 512],
                        start=(occ == 0), stop=(occ == 1))
                if jc == 0:
                    nc.vector.tensor_copy(ys[:, 0:512], py)
                else:
                    nc.scalar.copy(ys[:, 512:1024], py)
            # mid-kernel: keep DMA dispatch off the ACT queue (exp stream);
            # tail: use all three queues to drain fast.
            engs = ((nc.sync, nc.gpsimd, nc.scalar) if tail
                    else (nc.sync, nc.gpsimd))
            engs[i % len(engs)].dma_start(y_d[i * 128:(i + 1) * 128, :], ys)
            yield

    with tc.tile_pool(name="xTp", bufs=2) as xT_p, \
         tc.tile_pool(name="ppr", bufs=2, space="PSUM") as ppr_p, \
         tc.tile_pool(name="pss", bufs=2, space="PSUM") as pss_p, \
         tc.tile_pool(name="pso", bufs=1, space="PSUM") as pso_p, \
         tc.tile_pool(name="ptile", bufs=4) as pt_p, \
         tc.tile_pool(name="rrp", bufs=2) as rr_p, \
         tc.tile_pool(name="rbp", bufs=2) as rb_p, \
         tc.tile_pool(name="ystg", bufs=2) as ystg_p:

        xT_tiles = {}

        def load_x(c):
            xt = xT_p.tile([128, 8 * 512], BF16, name=f"xT{c}", tag="xT")
            x4 = xt.rearrange("p (kk t) -> p kk t", t=512)
            src = x3[:, :, c * 512:(c + 1) * 512]
            # chunk 0 loads before any exp traffic — the ACT queue is free;
            # later chunks keep DMA dispatch off the exp stream.
            eng2 = nc.scalar if c == 0 else nc.gpsimd
            nc.sync.dma_start(x4[:, 0:4, :], src[:, 0:4, :])
            eng2.dma_start(x4[:, 4:8, :], src[:, 4:8, :])
            xT_tiles[c] = xt

        def qk_unit(c, oc, w_sb, dest, bias):
            xt = xT_tiles[c]
            ps = ppr_p.tile([128, 512], f32, name="ps_qk", tag="ps")
            for kk in range(8):
                nc.tensor.matmul(
                    ps,
                    w_sb[kk][:, oc * 128:(oc + 1) * 128],
                    xt[:, kk * 512:(kk + 1) * 512],
                    start=(kk == 0), stop=(kk == 7))
            dst = dest[oc][:, c * 512:(c + 1) * 512]
            if bias is not None:
                nc.vector.tensor_scalar_add(dst, ps, bias[:, oc:oc + 1])
            else:
                nc.vector.tensor_copy(dst, ps)

        def v_unit(c, a):
            xt = xT_tiles[c]
            ps = ppr_p.tile([128, O], f32, name="ps_v", tag="ps")
            for kk in range(8):
                nc.tensor.matmul(
                    ps,
                    xt[:, kk * 512 + a * 128:kk * 512 + (a + 1) * 128],
                    wv_sb[kk],
                    start=(kk == 0), stop=(kk == 7))
            for h in range(HPC):
                nc.vector.tensor_copy(
                    vv[c * 4 + a][:, h * (S + 1):h * (S + 1) + S],
                    ps[:, h * S:(h + 1) * S])

        def gen_proj(c):
            """8 units in dependency-useful order: Q oc0, K oc0, V a0..a3,
            Q oc1, K oc1 — everything attn(c, 0) reads comes first, the oc1
            projections (needed only mid-chunk by attn(c, 1)) last.
            Emission order IS dependency order in Tile: a unit must be
            emitted before any unit that reads its output."""
            qk_unit(c, 0, wq_sb, qT, bq_sb)
            yield
            qk_unit(c, 0, wk_sb, kT, None)
            yield
            for a in range(4):
                v_unit(c, a)
                yield
            qk_unit(c, 1, wq_sb, qT, bq_sb)
            yield
            qk_unit(c, 1, wk_sb, kT, None)
            yield

        tm3v = trimask.rearrange("p (e j) -> p e j", e=2)

        def gen_attn(c, oc):
            """Software-pipelined r-loop: S(r)+exp(r) then AV(r-1)."""
            nr = 4 * (c + 1)
            po = [pso_p.tile([S + 1, 512], f32, name=f"po{e}",
                             tag=f"po{e}") for e in range(2)]
            pts = {}

            def s_exp(r):
                m = r - 4 * c
                j0 = 128 * m if m > 0 else 0
                ps = pss_p.tile([128, 1024], f32, name="ps_s", tag="pss")
                for e in range(2):
                    hb = e * 64
                    nc.tensor.matmul(
                        ps[:, e * 512 + j0:(e + 1) * 512],
                        kT[oc][hb:hb + 64, r * 128:(r + 1) * 128],
                        qT[oc][hb:hb + 64, c * 512 + j0:(c + 1) * 512],
                        start=True, stop=True)
                pt = pt_p.tile([128, 1024], BF16, name="pt_exp", tag="pt")
                ps3 = ps.rearrange("p (e j) -> p e j", e=2)[:, :, j0:]
                pt3 = pt.rearrange("p (e j) -> p e j", e=2)[:, :, j0:]
                nc.scalar.activation(pt3, ps3, AF.Exp, scale=inv_scale)
                if m >= 0:
                    nc.vector.tensor_mul(
                        pt3[:, :, 0:128], pt3[:, :, 0:128], tm3v)
                pts[r] = (pt, j0)

            def av(r):
                pt, j0 = pts.pop(r)
                for e in range(2):
                    h = 2 * oc + e
                    nc.tensor.matmul(
                        po[e][:, j0:],
                        vv[r][:, h * (S + 1):(h + 1) * (S + 1)],
                        pt[:, e * 512 + j0:(e + 1) * 512],
                        start=(r == 0), stop=(r == nr - 1))

            s_exp(0)
            yield
            for r in range(1, nr):
                s_exp(r)
                av(r - 1)
                yield
            av(nr - 1)
            # normalize + evict straight into oT (bf16)
            for e in range(2):
                # reciprocal_approx_fast reads the wrong partition for a
                # PSUM src with nonzero base, and plain InstReciprocal is
                # ~5 cyc/elem on HW (too slow for this chain) — so stage
                # the denominator row to SBUF, then fast-reciprocal there.
                rr0 = rr_p.tile([1, 512], f32, name="rr0", tag=f"rr0{e}")
                nc.vector.tensor_copy(rr0, po[e][S:S + 1, :])
                rr = rr_p.tile([1, 512], f32, name="rr", tag=f"rr{e}")
                nc.vector.reciprocal_approx_fast(rr, rr0)
                rb = rb_p.tile([64, 512], f32, name="rb", tag=f"rb{e}")
                nc.gpsimd.partition_broadcast(rb, rr)
                hb = e * 64
                nc.vector.tensor_mul(
                    oT[oc][hb:hb + 64, c * 512:(c + 1) * 512],
                    po[e][0:S, :], rb)
                yield

        _SENT = object()

        def weave(backbone, fillers, every=3):
            """Emit backbone units, inserting one filler unit (round-robin)
            every `every` backbone units, then drain filler leftovers."""
            n = 0
            fi = 0
            for _ in backbone:
                n += 1
                if n % every == 0 and fillers:
                    fi %= len(fillers)
                    if next(fillers[fi], _SENT) is _SENT:
                        fillers.pop(fi)
                    else:
                        fi += 1
            for f in fillers:
                for _ in f:
                    pass

        # table preload: dummy exp while weights/x load
        nc.scalar.activation(scratch, scratch, AF.Exp, scale=1.0)

        load_x(0)
        g0 = gen_proj(0)
        for _ in range(6):
            next(g0)  # Q oc0, K oc0, V a0..a3 — all attn(0, 0) reads
        load_x(1)
        for c in range(4):
            if 1 <= c <= 2:
                load_x(c + 1)
            fillers = []
            n_fill = 0
            if c == 0:
                fillers.append(g0)  # Q oc1, K oc1 for attn(0, 1)
                n_fill += 2
            if c + 1 < 4:
                fillers.append(gen_proj(c + 1))
                n_fill += 8
            if c > 0:
                fillers.append(gen_outp(c - 1, ppr_p, ystg_p))
                n_fill += 4
            n_back = 2 * (4 * (c + 1) + 2)

            def backbone(c=c):
                yield from gen_attn(c, 0)
                yield from gen_attn(c, 1)

            weave(backbone(), fillers, every=max(1, n_back // (n_fill + 1)))

    # Last chunk's output projection runs after the attention pools close:
    # 4 psum banks + a deeper stage ring drain the last 4 tiles with
    # maximum overlap.
    with tc.tile_pool(name="tailps", bufs=4, space="PSUM") as tail_ps, \
         tc.tile_pool(name="tailys", bufs=3) as tail_ys:
        for _ in gen_outp(3, tail_ps, tail_ys, ps_tag="tps", tail=True):
            pass


def build_program(loop_n=0):
    nc = bacc.Bacc("TRN2", target_bir_lowering=False, debug=False,
                   num_devices=NCORES)
    d = {
        "x": nc.dram_tensor("xT", [K, T], BF16, kind="ExternalInput").ap(),
        "wqT": nc.dram_tensor("wqT", [K, O], BF16, kind="ExternalInput").ap(),
        "wkT": nc.dram_tensor("wkT", [K, O], BF16, kind="ExternalInput").ap(),
        "wvT": nc.dram_tensor("wvT", [K, O], BF16, kind="ExternalInput").ap(),
        "woT": nc.dram_tensor("woT", [O, K], BF16, kind="ExternalInput").ap(),
        "bq": nc.dram_tensor("bq", [O], F32, kind="ExternalInput").ap(),
        "y": nc.dram_tensor("y", [T, K], F32, kind="ExternalOutput").ap(),
    }
    with tile.TileContext(nc) as tc:
        _build_body(nc, tc, d, loop_n=loop_n)
    nc.compile()
    return nc


def _get_program():
    if "nc" not in _CACHE:
        _CACHE["nc"] = build_program()
    return _CACHE["nc"]


def make_in_maps(x, Wq_w, Wk_w, Wv_w, Wo_w, Wq_b, Wk_b, Wv_b):
    in_maps = []
    for core in range(NCORES):
        b, hg = divmod(core, 4)
        sl = slice(hg * O, (hg + 1) * O)
        in_maps.append({
            "xT": np.ascontiguousarray(x[b].T).astype(NPBF16),
            "wqT": np.ascontiguousarray(Wq_w[sl, :].T).astype(NPBF16),
            "wkT": np.ascontiguousarray(Wk_w[sl, :].T).astype(NPBF16),
            "wvT": np.ascontiguousarray(Wv_w[sl, :].T).astype(NPBF16),
            "woT": np.ascontiguousarray(Wo_w[:, sl].T).astype(NPBF16),
            "bq": np.ascontiguousarray(Wq_b[sl], np.float32),
        })
    return in_maps


def _combine(results, Wv_b, Wo_w, Wo_b):
    # V bias passes through softmax exactly (rows sum to 1): add its
    # projected row on the host together with the output bias.
    bias_row = (np.asarray(Wo_b, np.float32)
                + np.asarray(Wv_b, np.float32) @ np.asarray(Wo_w, np.float32).T)
    y = np.empty((B, T, K), np.float32)
    for b in range(B):
        acc = results[b * 4]["y"].copy()
        for hg in range(1, 4):
            acc += results[b * 4 + hg]["y"]
        y[b] = acc + bias_row
    return y


def kernel(x, Wq_w, Wq_b, Wk_w, Wk_b, Wv_w, Wv_b, Wo_w, Wo_b):
    x = np.asarray(x, np.float32)
    nc = _get_program()
    in_maps = make_in_maps(x, np.asarray(Wq_w), np.asarray(Wk_w),
                           np.asarray(Wv_w), np.asarray(Wo_w),
                           np.asarray(Wq_b), np.asarray(Wk_b),
                           np.asarray(Wv_b))
    out = run_bass_kernel_spmd(nc, in_maps, list(range(NCORES)))
    return _combine(out.results, Wv_b, Wo_w, Wo_b)
